# revision 32
# baseline (speedup 1.0000x reference)
"""Trainium2 Bass kernel for nn_PraxisScatter (moe_routing) — v5.

Strategy (8 NeuronCores):
  - gate1 tensor-parallel over H (512 rows/core), 3-term fp16-hi + fp8
    cross corrections at PSUM scale 2^15; drains fp32 g.
  - g AllGathered in 3 packed chunks (m0 | m1 | m2+m3), each ONE
    collective carrying fp16 hi + bit-packed fp8 lo-residual.  The first
    chunk rides the cross-core rendezvous.
  - gate2 tensor-parallel 3-term fp16-hi + fp8 DR crosses, k-tiles in
    AG-chunk order; w2 host-permuted to match.  PSUM drains straight to
    y16 = (score-0.361)*64 fp16 (bias folded), so the score exchange is
    a 1MB fp16 AllToAll and needs no receive-side conversion.
  - up projections fp16 single-term; hc+hd exchanged in ONE combined
    AllToAll (fp16); gelu(hc) and gelu(hc+hd) precomputed during the
    score-exchange wait so the post-threshold tail is select+down only.
  - threshold via fixed-slope Newton on exact fp16 counts (vector+scalar
    halves) with a fp32 ones-matmul partition reduce+broadcast; PE
    re-warm burst during the search keeps the down matmuls at full clock.
  - fp16 down projection; weights prefetched during gate2/search.
"""

import sys

try:
    import concourse  # noqa: F401
except ImportError:  # pragma: no cover
    sys.path.insert(0, "/opt/trn_rl_repo")

import contextlib

import ml_dtypes
import numpy as np

import concourse.bass as bass  # noqa: F401
import concourse.mybir as mybir
import concourse.tile as tile
from concourse import bacc
from concourse.bass_utils import run_bass_kernel_spmd

BF16 = ml_dtypes.bfloat16
F16 = np.float16
F32 = np.float32
FP8 = ml_dtypes.float8_e4m3

NCORES = 8
B, S, D, H = 8, 128, 1024, 4096
T = B * S              # 1024 tokens
HS = H // NCORES       # 512 h rows per core
KT = H // 128          # 32 k-tiles over the full H
K_SEL = 256 * S        # 32768
Y_OFF, Y_SCL = 0.361, 64.0
C_NEWTON = 1.0 / 4260.0
R_ITER = 3
N_DW_PRE = 20          # dw tiles prefetched during gate2/search

f32 = mybir.dt.float32
bf16 = mybir.dt.bfloat16
fp16 = mybir.dt.float16
fp8e4 = mybir.dt.float8e4
AF = mybir.ActivationFunctionType
OP = mybir.AluOpType
DR = mybir.MatmulPerfMode.DoubleRow

# gate2 k-tile order (same on every core): AG chunk0 (every core's m0+m1),
# chunk1 (m2+m3).  k-tile kt covers global h rows kt*128..
KT_ORDER = ([4 * c + i for c in range(NCORES) for i in (0, 1)]
            + [4 * c + i for c in range(NCORES) for i in (2, 3)])


def _ag_pos(kt):
    """(hi chunk j, hi-row) and lo-row (always chunk1) of k-tile kt."""
    c, i = kt // 4, kt % 4
    if i < 2:
        return 0, c * 256 + i * 128, c * 512 + 256 + i * 64
    return 1, c * 512 + (i - 2) * 128, c * 512 + 256 + i * 64


def _build():
    nc = bacc.Bacc("TRN2", target_bir_lowering=False, debug=False,
                   num_devices=NCORES)

    xh_d = nc.dram_tensor("xh16", [D, T], fp16, kind="ExternalInput").ap()
    x8a_d = nc.dram_tensor("x8a", [4, 128, 2, T], fp8e4, kind="ExternalInput").ap()
    x8b_d = nc.dram_tensor("x8b", [4, 128, 2, T], fp8e4, kind="ExternalInput").ap()
    w1h_d = nc.dram_tensor("w1h16", [D, HS], fp16, kind="ExternalInput").ap()
    w1a_d = nc.dram_tensor("w1a8", [4, 128, 2, HS], fp8e4, kind="ExternalInput").ap()
    w1b_d = nc.dram_tensor("w1b8", [4, 128, 2, HS], fp8e4, kind="ExternalInput").ap()
    w2h_d = nc.dram_tensor("w2h16", [KT, 128, HS], fp16, kind="ExternalInput").ap()
    w2a_d = nc.dram_tensor("w2a8", [KT // 2, 128, 2, HS], fp8e4, kind="ExternalInput").ap()
    w2b_d = nc.dram_tensor("w2b8", [KT // 2, 128, 2, HS], fp8e4, kind="ExternalInput").ap()
    upc_d = nc.dram_tensor("upc16", [D, HS], fp16, kind="ExternalInput").ap()
    upd_d = nc.dram_tensor("upd16", [D, HS], fp16, kind="ExternalInput").ap()
    dw_d = nc.dram_tensor("dwT16", [H, D], fp16, kind="ExternalInput").ap()
    b1_d = nc.dram_tensor("b1s", [4, 128], f32, kind="ExternalInput").ap()
    b2y_d = nc.dram_tensor("b2ys", [4, 128], f32, kind="ExternalInput").ap()
    bc_d = nc.dram_tensor("bcs", [4, 128], f32, kind="ExternalInput").ap()
    bd_d = nc.dram_tensor("bds", [4, 128], f32, kind="ExternalInput").ap()
    dbias_d = nc.dram_tensor("dbias", [128, D], f32, kind="ExternalInput").ap()
    out_d = nc.dram_tensor("out", [S, D], f32, kind="ExternalOutput").ap()

    # collective buffers
    g_ag_in = [nc.dram_tensor("g_ag_in0", [256, T], fp16).ap(),
               nc.dram_tensor("g_ag_in1", [512, T], fp16).ap()]
    g_ag_out = [nc.dram_tensor("g_ag_out0", [NCORES * 256, T], fp16,
                               addr_space="Shared").ap(),
                nc.dram_tensor("g_ag_out1", [NCORES * 512, T], fp16,
                               addr_space="Shared").ap()]
    h_a2a_in = nc.dram_tensor("h_a2a_in", [NCORES, 2, HS, S], fp16).ap()
    h_a2a_out = nc.dram_tensor("h_a2a_out", [NCORES, 2, HS, S], fp16).ap()
    y_a2a_in = nc.dram_tensor("y_a2a_in", [NCORES, HS, S], fp16).ap()
    y_a2a_out = nc.dram_tensor("y_a2a_out", [NCORES, HS, S], fp16).ap()

    rg = [list(range(NCORES))]

    with tile.TileContext(nc) as tc, contextlib.ExitStack() as ctx:
        en = tc.nc
        const = ctx.enter_context(tc.tile_pool(name="const", bufs=1))
        xp = ctx.enter_context(tc.tile_pool(name="xres", bufs=1))
        w2p = ctx.enter_context(tc.tile_pool(name="w2p", bufs=5))
        gkp = ctx.enter_context(tc.tile_pool(name="gkp", bufs=2))
        g8p = ctx.enter_context(tc.tile_pool(name="g8p", bufs=5))
        gsp = ctx.enter_context(tc.tile_pool(name="gsp", bufs=2))
        drain = ctx.enter_context(tc.tile_pool(name="drain", bufs=2))
        big = ctx.enter_context(tc.tile_pool(name="big", bufs=1))
        dwp = ctx.enter_context(tc.tile_pool(name="dwp", bufs=N_DW_PRE + 2))
        ps = ctx.enter_context(tc.tile_pool(name="ps", bufs=8, space="PSUM"))

        _cc_prev = [None]

        def cc(kind, ins, outs, waits=()):
            h = en.gpsimd.collective_compute(kind, OP.bypass, ins=ins,
                                             outs=outs, replica_groups=rg)
            for w in waits:
                tile.add_dep_helper(h.ins, w.ins,
                                    reason="collective input writer")
            if _cc_prev[0] is not None:
                tile.add_dep_helper(h.ins, _cc_prev[0].ins,
                                    reason="collective issue-order chain")
            _cc_prev[0] = h
            return h

        # ---------- loads (xh/w1 interleaved per k for earliest start) ----
        xh_s = xp.tile([128, 8, T], fp16, tag="xh")
        w1_s = xp.tile([128, 8, HS], fp16, tag="w1")
        for k in range(8):
            en.sync.dma_start(xh_s[:, k], xh_d[k * 128:(k + 1) * 128])
            en.sync.dma_start(w1_s[:, k], w1h_d[k * 128:(k + 1) * 128])
        x8a_s = xp.tile([128, 4, 2, T], fp8e4, tag="x8a")
        en.scalar.dma_start(x8a_s[:], x8a_d.rearrange("a p l t -> p a l t"))
        x8b_s = xp.tile([128, 4, 2, T], fp8e4, tag="x8b")
        en.scalar.dma_start(x8b_s[:], x8b_d.rearrange("a p l t -> p a l t"))
        w1a_s = xp.tile([128, 4, 2, HS], fp8e4, tag="w1a")
        en.scalar.dma_start(w1a_s[:], w1a_d.rearrange("a p l m -> p a l m"))
        w1b_s = xp.tile([128, 4, 2, HS], fp8e4, tag="w1b")
        en.scalar.dma_start(w1b_s[:], w1b_d.rearrange("a p l m -> p a l m"))
        b1_s = const.tile([128, 4], f32, tag="b1")
        en.sync.dma_start(b1_s[:], b1_d.rearrange("m p -> p m"))
        b2y_s = const.tile([128, 4], f32, tag="b2y")
        en.sync.dma_start(b2y_s[:], b2y_d.rearrange("m p -> p m"))
        bc_s = const.tile([128, 4], f32, tag="bc")
        en.sync.dma_start(bc_s[:], bc_d.rearrange("m p -> p m"))
        bd_s = const.tile([128, 4], f32, tag="bd")
        en.sync.dma_start(bd_s[:], bd_d.rearrange("m p -> p m"))
        dbias_s = const.tile([128, D], f32, tag="dbias")
        en.sync.dma_start(dbias_s[:], dbias_d[:])

        N0, N1 = slice(0, 512), slice(512, 1024)

        # ---------- gate1: hi(m0,m1) -> AG0; hi(m2,m3)+all lo -> AG1 ------
        ag0_wr, ag1_wr = [], []
        for m in range(4):
            mslc = slice(m * 128, (m + 1) * 128)
            p0 = ps.tile([128, 512], f32, tag="ps", name=f"g1_{m}_0")
            p1 = ps.tile([128, 512], f32, tag="ps", name=f"g1_{m}_1")
            for k in range(8):
                w = w1_s[:, k, mslc]
                en.tensor.matmul(p0[:], w, xh_s[:, k, N0],
                                 start=(k == 0), stop=False)
                en.tensor.matmul(p1[:], w, xh_s[:, k, N1],
                                 start=(k == 0), stop=False)
            for a in range(4):
                wa = w1a_s[:, a, :, mslc]
                wb = w1b_s[:, a, :, mslc]
                en.tensor.matmul(p0[:], wa, x8a_s[:, a, :, N0],
                                 start=False, stop=False, perf_mode=DR)
                en.tensor.matmul(p1[:], wa, x8a_s[:, a, :, N1],
                                 start=False, stop=False, perf_mode=DR)
                en.tensor.matmul(p0[:], wb, x8b_s[:, a, :, N0],
                                 start=False, stop=(a == 3), perf_mode=DR)
                en.tensor.matmul(p1[:], wb, x8b_s[:, a, :, N1],
                                 start=False, stop=(a == 3), perf_mode=DR)
            gf = drain.tile([128, T], f32, tag="gf", name=f"gf{m}")
            en.scalar.activation(gf[:, N0], p0[:], AF.Relu,
                                 bias=b1_s[:, m:m + 1], scale=2.0 ** -15)
            en.scalar.activation(gf[:, N1], p1[:], AF.Relu,
                                 bias=b1_s[:, m:m + 1], scale=2.0 ** -15)
            g16 = gsp.tile([128, T], fp16, tag="g16", name=f"g16_{m}")
            en.vector.tensor_copy(g16[:], gf[:])
            glf = drain.tile([128, T], f32, tag="gf", name=f"glf{m}")
            en.vector.tensor_sub(glf[:], gf[:], g16[:])
            gl8 = gsp.tile([128, T], fp8e4, tag="gl8", name=f"gl8_{m}")
            en.vector.tensor_scalar(gl8[:], glf[:], 2.0 ** 12, None,
                                    op0=OP.mult)
            lo_dst = g_ag_in[1][256 + m * 64: 256 + (m + 1) * 64] \
                .bitcast(fp8e4).rearrange("a (b f) -> (a b) f", b=2)
            wl = en.sync.dma_start(lo_dst, gl8[:])
            if m < 2:
                wh = en.sync.dma_start(g_ag_in[0][m * 128:(m + 1) * 128],
                                       g16[:])
                ag0_wr += [wh]
                ag1_wr += [wl]
                if m == 1:
                    cc("AllGather", [g_ag_in[0][:]], [g_ag_out[0][:]],
                       waits=tuple(ag0_wr))
            else:
                wh = en.sync.dma_start(
                    g_ag_in[1][(m - 2) * 128:(m - 1) * 128], g16[:])
                ag1_wr += [wh, wl]
                if m == 3:
                    cc("AllGather", [g_ag_in[1][:]], [g_ag_out[1][:]],
                       waits=tuple(ag1_wr))

        # ---------- up projections (fp16, combined a2a) ----------
        up_s = {}
        for nm, src in (("c", upc_d), ("d", upd_d)):
            u = xp.tile([128, 8, HS], fp16, tag=f"up{nm}")
            en.sync.dma_start(u[:], src.rearrange("(ko p) m -> p ko m", p=128))
            up_s[nm] = u
        h_wr = []
        h_in_v = h_a2a_in.rearrange("j b (p a2) s -> p b a2 j s", a2=4)
        for bi, (nm, bias_t) in enumerate((("c", bc_s), ("d", bd_s))):
            for m in range(4):
                mslc = slice(m * 128, (m + 1) * 128)
                p0 = ps.tile([128, 512], f32, tag="ps", name=f"u{nm}_{m}_0")
                p1 = ps.tile([128, 512], f32, tag="ps", name=f"u{nm}_{m}_1")
                for k in range(8):
                    w = up_s[nm][:, k, mslc]
                    en.tensor.matmul(p0[:], w, xh_s[:, k, N0],
                                     start=(k == 0), stop=(k == 7))
                    en.tensor.matmul(p1[:], w, xh_s[:, k, N1],
                                     start=(k == 0), stop=(k == 7))
                hq = drain.tile([128, T], fp16, tag="hq", name=f"hq{nm}{m}")
                en.scalar.activation(hq[:, N0], p0[:], AF.Identity,
                                     bias=bias_t[:, m:m + 1])
                en.scalar.activation(hq[:, N1], p1[:], AF.Identity,
                                     bias=bias_t[:, m:m + 1])
                h_wr.append(en.sync.dma_start(h_in_v[:, bi, m], hq[:]))
        h_cc = cc("AllToAll", [h_a2a_in[:]], [h_a2a_out[:]],
                  waits=tuple(h_wr))

        # ---------- gate2: k-tiles in AG order ----------
        pts = {(m, n): ps.tile([128, 512], f32, tag="ps", name=f"g2_{m}_{n}")
               for m in range(4) for n in range(2)}
        n_dw = [0]
        dw_tiles = {}

        def issue_dw(n=1):
            for _ in range(n):
                i = n_dw[0]
                if i >= KT:
                    return
                n_dw[0] += 1
                dwk = dwp.tile([128, D], fp16, tag="dwk", name=f"dwk{i}")
                en.scalar.dma_start(dwk[:], dw_d[i * 128:(i + 1) * 128])
                dw_tiles[i] = dwk

        # persistent fp8(g16*0.5) for chunk0 k-tiles lives in the dead
        # xh tile (16 x [128,1024] fp8 = its exact byte size)
        g8a_all = xh_s[:].bitcast(fp8e4).rearrange("p a t -> p (a t)")

        # phase A: chunk0 hi-terms only (lo residuals arrive with AG1)
        for idx in range(16):
            kt = KT_ORDER[idx]
            j, hi_row, lo_row = _ag_pos(kt)
            gk = gkp.tile([128, T], fp16, tag="gk", name=f"gkA{idx}")
            en.sync.dma_start(gk[:], g_ag_out[j][hi_row:hi_row + 128])
            en.scalar.activation(
                g8a_all[:, idx * T:(idx + 1) * T], gk[:],
                AF.Identity, scale=0.5)
            w2k = w2p.tile([128, HS], fp16, tag="w2k", name=f"w2k{idx}")
            en.sync.dma_start(w2k[:], w2h_d[idx])
            first = (idx == 0)
            for m in range(4):
                mslc = slice(m * 128, (m + 1) * 128)
                en.tensor.matmul(pts[(m, 0)][:], w2k[:, mslc],
                                 gk[:, N0], start=first, stop=False)
                en.tensor.matmul(pts[(m, 1)][:], w2k[:, mslc],
                                 gk[:, N1], start=first, stop=False)

        # phase B: chunk1 hi + crosses (current-style pair loop)
        for a in range(8, KT // 2):
            g8ap = g8p.tile([128, 2, T], fp8e4, tag="g8a", name=f"g8a{a}")
            g8bp = g8p.tile([128, 2, T], fp8e4, tag="g8b", name=f"g8b{a}")
            gk2 = gkp.tile([128, 2, T], fp16, tag="gk2", name=f"gk{a}")
            for half in range(2):
                idx = 2 * a + half
                kt = KT_ORDER[idx]
                j, hi_row, lo_row = _ag_pos(kt)
                en.sync.dma_start(gk2[:, half],
                                  g_ag_out[j][hi_row:hi_row + 128])
                lo_src = g_ag_out[1][lo_row:lo_row + 64] \
                    .bitcast(fp8e4).rearrange("a (b f) -> (a b) f", b=2)
                en.sync.dma_start(g8bp[:, half], lo_src)
                en.scalar.activation(g8ap[:, half], gk2[:, half],
                                     AF.Identity, scale=0.5)
                w2k = w2p.tile([128, HS], fp16, tag="w2k", name=f"w2k{idx}")
                en.sync.dma_start(w2k[:], w2h_d[idx])
                for m in range(4):
                    mslc = slice(m * 128, (m + 1) * 128)
                    en.tensor.matmul(pts[(m, 0)][:], w2k[:, mslc],
                                     gk2[:, half, N0], start=False,
                                     stop=False)
                    en.tensor.matmul(pts[(m, 1)][:], w2k[:, mslc],
                                     gk2[:, half, N1], start=False,
                                     stop=False)
            w2ak = w2p.tile([128, 2, HS], fp8e4, tag="w2c", name=f"w2a{a}")
            en.sync.dma_start(w2ak[:], w2a_d[a])
            w2bk = w2p.tile([128, 2, HS], fp8e4, tag="w2c", name=f"w2b{a}")
            en.sync.dma_start(w2bk[:], w2b_d[a])
            for m in range(4):
                mslc = slice(m * 128, (m + 1) * 128)
                en.tensor.matmul(pts[(m, 0)][:], w2ak[:, :, mslc],
                                 g8ap[:, :, N0], start=False, stop=False,
                                 perf_mode=DR)
                en.tensor.matmul(pts[(m, 1)][:], w2ak[:, :, mslc],
                                 g8ap[:, :, N1], start=False, stop=False,
                                 perf_mode=DR)
                en.tensor.matmul(pts[(m, 0)][:], w2bk[:, :, mslc],
                                 g8bp[:, :, N0], start=False, stop=False,
                                 perf_mode=DR)
                en.tensor.matmul(pts[(m, 1)][:], w2bk[:, :, mslc],
                                 g8bp[:, :, N1], start=False, stop=False,
                                 perf_mode=DR)
            if a >= 10 and a % 2 == 0:
                issue_dw(2)

        # phase C: chunk0 crosses (persisted g8a + streamed lo)
        for a in range(8):
            g8bp = g8p.tile([128, 2, T], fp8e4, tag="g8b", name=f"g8bC{a}")
            for half in range(2):
                idx = 2 * a + half
                kt = KT_ORDER[idx]
                j, hi_row, lo_row = _ag_pos(kt)
                lo_src = g_ag_out[1][lo_row:lo_row + 64] \
                    .bitcast(fp8e4).rearrange("a (b f) -> (a b) f", b=2)
                en.sync.dma_start(g8bp[:, half], lo_src)
            g8av = g8a_all[:, 2 * a * T:(2 * a + 2) * T] \
                .rearrange("p (l t) -> p l t", l=2)
            w2ak = w2p.tile([128, 2, HS], fp8e4, tag="w2c", name=f"w2aC{a}")
            en.sync.dma_start(w2ak[:], w2a_d[a])
            w2bk = w2p.tile([128, 2, HS], fp8e4, tag="w2c", name=f"w2bC{a}")
            en.sync.dma_start(w2bk[:], w2b_d[a])
            last = (a == 7)
            for m in range(4):
                mslc = slice(m * 128, (m + 1) * 128)
                en.tensor.matmul(pts[(m, 0)][:], w2ak[:, :, mslc],
                                 g8av[:, :, N0], start=False, stop=False,
                                 perf_mode=DR)
                en.tensor.matmul(pts[(m, 1)][:], w2ak[:, :, mslc],
                                 g8av[:, :, N1], start=False, stop=False,
                                 perf_mode=DR)
                en.tensor.matmul(pts[(m, 0)][:], w2bk[:, :, mslc],
                                 g8bp[:, :, N0], start=False, stop=last,
                                 perf_mode=DR)
                en.tensor.matmul(pts[(m, 1)][:], w2bk[:, :, mslc],
                                 g8bp[:, :, N1], start=False, stop=last,
                                 perf_mode=DR)
            if a % 2 == 0:
                issue_dw(2)

        # drains straight to y16 = (score - Y_OFF)*Y_SCL in fp16.
        # a2a block rows are (p, a2)-ordered so the receive side fills with
        # one contiguous-per-partition DMA.
        y_wr = []
        y_in_v = y_a2a_in.rearrange("j (p a2) s -> p a2 j s", a2=4)
        for m in range(4):
            ym = drain.tile([128, T], fp16, tag="hq", name=f"ym{m}")
            en.scalar.activation(ym[:, N0], pts[(m, 0)][:], AF.Identity,
                                 bias=b2y_s[:, m:m + 1], scale=2.0 ** -9)
            en.scalar.activation(ym[:, N1], pts[(m, 1)][:], AF.Identity,
                                 bias=b2y_s[:, m:m + 1], scale=2.0 ** -9)
            y_wr.append(en.sync.dma_start(y_in_v[:, m], ym[:]))
        y_cc = cc("AllToAll", [y_a2a_in[:]], [y_a2a_out[:]],
                  waits=tuple(y_wr))
        issue_dw(N_DW_PRE - n_dw[0])

        # ---------- h fills (wait h a2a) + gelu precompute ----------
        hc_s = big.tile([128, KT, S], fp16, tag="hc", name="hc_s")
        hd_s = big.tile([128, KT, S], fp16, tag="hd", name="hd_s")
        gp_s = big.tile([128, KT, S], fp16, tag="gp", name="gp_s")
        h_src = h_a2a_out.rearrange("r b (p a2) s -> p b r (a2 s)", a2=4)
        for bi, dst in ((0, hc_s), (1, hd_s)):
            hr = en.sync.dma_start(
                dst[:].rearrange("p (r a2) s -> p r (a2 s)", a2=4),
                h_src[:, bi])
            tile.add_dep_helper(hr.ins, h_cc.ins, reason="h a2a done")
        # gp := hc + hd; gc := gelu(hc) -> hd_s; gelu(gp) -> hc_s;
        # gd := gelu(hp) - gc -> gp_s.  (runs during the y a2a wait)
        en.vector.tensor_tensor(gp_s[:], hc_s[:], hd_s[:], op=OP.add)
        en.scalar.activation(hd_s[:], hc_s[:], AF.Gelu)
        en.scalar.activation(hc_s[:], gp_s[:], AF.Gelu)
        en.vector.tensor_sub(gp_s[:], hc_s[:], hd_s[:])
        a_s = hc_s        # gelu(hp) content is dead once gd is computed

        # PE warm-keeper: a short matmul burst after gate2 drains plus a
        # couple per search round keeps the HAM clock gate open so the
        # down projection runs at full rate (outputs unused)
        pwm = ps.tile([128, 512], f32, tag="ps", name="pwm")

        def warm_mms(n):
            for _ in range(n):
                en.tensor.matmul(pwm[:], up_s["c"][:, 0, 0:128],
                                 up_s["c"][:, 0, :], start=True, stop=True)

        warm_mms(8)

        # ---------- y16 fill (two parallel DMA queues) ----------
        y16 = big.tile([128, KT, S], fp16, tag="y16", name="y16")
        yfills = []
        for hh, eng in ((0, en.sync), (1, en.scalar)):
            yf = eng.dma_start(
                y16[:, hh * 16:(hh + 1) * 16, :]
                .rearrange("p (r a2) s -> p r (a2 s)", a2=4),
                y_a2a_out[hh * 4:(hh + 1) * 4]
                .rearrange("r (p a2) s -> p r (a2 s)", a2=4))
            tile.add_dep_helper(yf.ins, y_cc.ins, reason="y a2a done")
            yfills.append(yf)

        # ---------- Newton threshold search on y16 ----------
        y_flat = y16.rearrange("p a b -> p (a b)")
        half = (KT * S) // 2
        # scratch targets for the count passes: carve them out of the
        # long-dead xh tile (count writes are garbage, only accum matters)
        cmpb = w1_s[:, 0:2, :].bitcast(fp8e4).rearrange("p a t -> p (a t)")
        sgnb = w1_s[:, 4:6, :].bitcast(fp8e4).rearrange("p a t -> p (a t)")
        ones32 = const.tile([128, 128], f32, tag="ones32", name="ones32")
        en.vector.memset(ones32[:], 1.0)
        yt = const.tile([128, 1], f32, tag="yt", name="yt")
        nyt = const.tile([128, 1], f32, tag="nyt", name="nyt")
        uu = const.tile([128, 1], f32, tag="uu", name="uu")
        en.vector.memset(yt[:], 0.0)
        en.vector.memset(nyt[:], 0.0)
        accs = const.tile([128, 2], f32, tag="accs", name="accs")
        rmax = const.tile([128, KT], f32, tag="rmax", name="rmax")

        for r in range(R_ITER):
            hv = en.vector.tensor_scalar(cmpb, y_flat[:, 0:half],
                                         yt[:], 0.0, op0=OP.is_gt,
                                         op1=OP.add, accum_out=accs[:, 0:1])
            hs = en.scalar.activation(sgnb, y_flat[:, half:],
                                      AF.Sign, bias=nyt[:],
                                      accum_out=accs[:, 1:2])
            if r == 0:
                for dep in yfills:
                    tile.add_dep_helper(hv.ins, dep.ins, reason="y16 ready")
                    tile.add_dep_helper(hs.ins, dep.ins, reason="y16 ready")
            en.vector.scalar_tensor_tensor(uu[:], accs[:, 1:2], 0.5,
                                           accs[:, 0:1],
                                           op0=OP.mult, op1=OP.add)
            en.vector.tensor_scalar(uu[:], uu[:], 768.0, None, op0=OP.add)
            pred = ps.tile([128, 1], f32, tag="ps", name=f"pred{r}")
            en.tensor.matmul(pred[:], ones32[:], uu[:],
                             start=True, stop=True)
            yt_h = en.vector.scalar_tensor_tensor(yt[:], pred[:], C_NEWTON,
                                                  yt[:],
                                                  op0=OP.mult, op1=OP.add)
            if r == 0:
                round0_yt = yt_h
            en.vector.tensor_scalar(nyt[:], yt[:], -1.0, None, op0=OP.mult)
            warm_mms(3)
            if r == 1:
                # rowmax (needed only for the final select) slots into
                # vector gaps between rounds; pin it after round 0 so the
                # scheduler cannot hoist it ahead of the first count
                for q in range(4):
                    hq_ = en.vector.reduce_max(rmax[:, q * 8:(q + 1) * 8],
                                               y16[:, q * 8:(q + 1) * 8, :],
                                               axis=mybir.AxisListType.X)
                    tile.add_dep_helper(hq_.ins, round0_yt.ins,
                                        reason="defer rowmax past round 0")

        sel = const.tile([128, KT], f32, tag="sel", name="sel")
        en.vector.tensor_scalar(sel[:], rmax[:], yt[:], None, op0=OP.is_gt)

        # ---------- select + down, pipelined per 8-ktile chunk ----------
        # a := gd*sel + gc  (gd in gp_s, gc in hd_s) -> a_s in hc_s
        pd0 = ps.tile([128, 512], f32, tag="ps", name="dn0")
        pd1 = ps.tile([128, 512], f32, tag="ps", name="dn1")
        for q in range(4):
            issue_dw(8)
            for kt in range(q * 8, (q + 1) * 8):
                en.vector.scalar_tensor_tensor(
                    a_s[:, kt, :], gp_s[:, kt, :], sel[:, kt:kt + 1],
                    hd_s[:, kt, :], op0=OP.mult, op1=OP.add)
            for kt in range(q * 8, (q + 1) * 8):
                dwk = dw_tiles[kt]
                en.tensor.matmul(pd0[:], a_s[:, kt, :], dwk[:, N0],
                                 start=(kt == 0), stop=(kt == KT - 1))
                en.tensor.matmul(pd1[:], a_s[:, kt, :], dwk[:, N1],
                                 start=(kt == 0), stop=(kt == KT - 1))
        osb = drain.tile([128, D], f32, tag="gf", name="osb")
        en.vector.tensor_tensor(osb[:, N0], pd0[:], dbias_s[:, N0], op=OP.add)
        en.sync.dma_start(out_d[:, N0], osb[:, N0])
        en.vector.tensor_tensor(osb[:, N1], pd1[:], dbias_s[:, N1], op=OP.add)
        en.sync.dma_start(out_d[:, N1], osb[:, N1])

    nc.compile()
    return nc


_NC_CACHE = None


def _f16hi(a, scale):
    """fp16(a*scale) and the fp32 residual a - fp16(a*scale)/scale."""
    hi = (a.astype(np.float64) * scale).astype(F16)
    res = (a.astype(np.float64) - hi.astype(np.float64) / scale).astype(F32)
    return hi, res


def _pair_k(a):
    """[n*256, X] -> [n, 128, 2, X] : (2t, 2t+1) k-tile pairs."""
    n = a.shape[0] // 256
    return np.ascontiguousarray(a.reshape(n, 2, 128, -1).transpose(0, 2, 1, 3))


def _prep_in_maps(x, w1, b1, w2, b2, upw, upb, ucw, ucb, dw, db):
    xt = np.ascontiguousarray(x.reshape(T, D).T).astype(F32)   # [D, T]
    xh16 = xt.astype(F16)
    x_lo = xt - xh16.astype(F32)
    x8a = _pair_k((xt * 0.5).astype(FP8))
    x8b = _pair_k((x_lo * (2.0 ** 9)).astype(FP8))
    udw = upw - ucw
    udb = upb - ucb
    dwT16 = np.ascontiguousarray(dw.T.astype(F16))             # [H, D]
    dbias = np.ascontiguousarray(np.tile(db[None, :], (128, 1)).astype(F32))

    perm = (np.asarray(KT_ORDER)[:, None] * 128
            + np.arange(128)[None, :]).reshape(-1)

    in_maps = []
    for c in range(NCORES):
        sh = slice(c * HS, (c + 1) * HS)
        w1t = np.ascontiguousarray(w1[sh].T).astype(F32)       # [D, HS]
        w1h, w1res = _f16hi(w1t, 2.0 ** 15)
        w2t = np.ascontiguousarray(w2[sh].T).astype(F32)       # [H, HS]
        w2t = np.ascontiguousarray(w2t[perm])
        w2h, w2res = _f16hi(w2t, 2.0 ** 15)
        b2y = (b2[sh].astype(F32) - Y_OFF) * Y_SCL
        in_maps.append({
            "xh16": xh16,
            "x8a": x8a,
            "x8b": x8b,
            "w1h16": w1h,
            "w1a8": _pair_k((w1res * (2.0 ** 16)).astype(FP8)),
            "w1b8": _pair_k((w1t * (2.0 ** 6)).astype(FP8)),
            "w2h16": np.ascontiguousarray(w2h.reshape(KT, 128, HS)),
            "w2a8": _pair_k((w2res * (2.0 ** 16)).astype(FP8)),
            "w2b8": _pair_k((w2t * (2.0 ** 3)).astype(FP8)),
            "upc16": np.ascontiguousarray(ucw[sh].T.astype(F16)),
            "upd16": np.ascontiguousarray(udw[sh].T.astype(F16)),
            "dwT16": dwT16,
            "b1s": np.ascontiguousarray(b1[sh].reshape(4, 128)).astype(F32),
            "b2ys": np.ascontiguousarray(b2y.reshape(4, 128)).astype(F32),
            "bcs": np.ascontiguousarray(ucb[sh].reshape(4, 128)).astype(F32),
            "bds": np.ascontiguousarray(udb[sh].reshape(4, 128)).astype(F32),
            "dbias": dbias,
        })
    return in_maps


def kernel_in_maps(**inputs):
    names = ["inputs", "gate_w1", "gate_b1", "gate_w2", "gate_b2",
             "up_prev_w", "up_prev_b", "up_curr_w", "up_curr_b",
             "down_w", "down_b"]
    vals = [np.asarray(inputs[n], F32) for n in names]
    return _prep_in_maps(*vals)


def kernel(**inputs):
    global _NC_CACHE
    if _NC_CACHE is None:
        _NC_CACHE = _build()
    nc = _NC_CACHE
    in_maps = kernel_in_maps(**inputs)
    res = run_bass_kernel_spmd(nc, in_maps, core_ids=list(range(NCORES)))
    out = np.stack([res.results[c]["out"] for c in range(NCORES)], axis=0)
    return np.ascontiguousarray(out.astype(F32))


# revision 34
# speedup vs baseline: 1.1229x; 1.1229x over previous
"""Trainium2 Bass kernel for nn_PraxisScatter (moe_routing) — v5.

Strategy (8 NeuronCores):
  - gate1 tensor-parallel over H (512 rows/core), 3-term fp16-hi + fp8
    cross corrections at PSUM scale 2^15; drains fp32 g.
  - g AllGathered in 3 packed chunks (m0 | m1 | m2+m3), each ONE
    collective carrying fp16 hi + bit-packed fp8 lo-residual.  The first
    chunk rides the cross-core rendezvous.
  - gate2 tensor-parallel 3-term fp16-hi + fp8 DR crosses, k-tiles in
    AG-chunk order; w2 host-permuted to match.  PSUM drains straight to
    y16 = (score-0.361)*64 fp16 (bias folded), so the score exchange is
    a 1MB fp16 AllToAll and needs no receive-side conversion.
  - up projections fp16 single-term; hc+hd exchanged in ONE combined
    AllToAll (fp16); gelu(hc) and gelu(hc+hd) precomputed during the
    score-exchange wait so the post-threshold tail is select+down only.
  - threshold via fixed-slope Newton on exact fp16 counts (vector+scalar
    halves) with a fp32 ones-matmul partition reduce+broadcast; PE
    re-warm burst during the search keeps the down matmuls at full clock.
  - fp16 down projection; weights prefetched during gate2/search.
"""

import sys

try:
    import concourse  # noqa: F401
except ImportError:  # pragma: no cover
    sys.path.insert(0, "/opt/trn_rl_repo")

import contextlib

import ml_dtypes
import numpy as np

import concourse.bass as bass  # noqa: F401
import concourse.mybir as mybir
import concourse.tile as tile
from concourse import bacc
from concourse.bass_utils import run_bass_kernel_spmd

BF16 = ml_dtypes.bfloat16
F16 = np.float16
F32 = np.float32
FP8 = ml_dtypes.float8_e4m3

NCORES = 8
B, S, D, H = 8, 128, 1024, 4096
T = B * S              # 1024 tokens
HS = H // NCORES       # 512 h rows per core
KT = H // 128          # 32 k-tiles over the full H
K_SEL = 256 * S        # 32768
Y_OFF, Y_SCL = 0.361, 64.0
C_NEWTON = 1.0 / 4260.0
R_ITER = 3
N_DW_PRE = 16          # dw tiles prefetched during gate2/search

f32 = mybir.dt.float32
bf16 = mybir.dt.bfloat16
fp16 = mybir.dt.float16
fp8e4 = mybir.dt.float8e4
AF = mybir.ActivationFunctionType
OP = mybir.AluOpType
DR = mybir.MatmulPerfMode.DoubleRow

# gate2 k-tile order (same on every core): AG chunk0 (every core's m0),
# chunk1 (m1), chunk2 (m2+m3).  k-tile kt covers global h rows kt*128..
KT_ORDER = ([4 * c for c in range(NCORES)]
            + [4 * c + 1 for c in range(NCORES)]
            + [4 * c + i for c in range(NCORES) for i in (2, 3)])


def _ag_pos(kt):
    """(chunk j, hi-row, lo-row) of k-tile kt inside g_ag_out[j]."""
    c, i = kt // 4, kt % 4
    if i < 2:
        return i, c * 192, c * 192 + 128
    return 2, c * 384 + (i - 2) * 128, c * 384 + 256 + (i - 2) * 64


def _build():
    nc = bacc.Bacc("TRN2", target_bir_lowering=False, debug=False,
                   num_devices=NCORES)

    xh_d = nc.dram_tensor("xh16", [D, T], fp16, kind="ExternalInput").ap()
    x8a_d = nc.dram_tensor("x8a", [4, 128, 2, T], fp8e4, kind="ExternalInput").ap()
    x8b_d = nc.dram_tensor("x8b", [4, 128, 2, T], fp8e4, kind="ExternalInput").ap()
    w1h_d = nc.dram_tensor("w1h16", [D, HS], fp16, kind="ExternalInput").ap()
    w1a_d = nc.dram_tensor("w1a8", [4, 128, 2, HS], fp8e4, kind="ExternalInput").ap()
    w1b_d = nc.dram_tensor("w1b8", [4, 128, 2, HS], fp8e4, kind="ExternalInput").ap()
    w2h_d = nc.dram_tensor("w2h16", [KT, 128, HS], fp16, kind="ExternalInput").ap()
    w2a_d = nc.dram_tensor("w2a8", [KT // 2, 128, 2, HS], fp8e4, kind="ExternalInput").ap()
    w2b_d = nc.dram_tensor("w2b8", [KT // 2, 128, 2, HS], fp8e4, kind="ExternalInput").ap()
    upc_d = nc.dram_tensor("upc16", [D, HS], fp16, kind="ExternalInput").ap()
    upd_d = nc.dram_tensor("upd16", [D, HS], fp16, kind="ExternalInput").ap()
    dw_d = nc.dram_tensor("dwT16", [H, D], fp16, kind="ExternalInput").ap()
    b1_d = nc.dram_tensor("b1s", [4, 128], f32, kind="ExternalInput").ap()
    b2y_d = nc.dram_tensor("b2ys", [4, 128], f32, kind="ExternalInput").ap()
    bc_d = nc.dram_tensor("bcs", [4, 128], f32, kind="ExternalInput").ap()
    bd_d = nc.dram_tensor("bds", [4, 128], f32, kind="ExternalInput").ap()
    dbias_d = nc.dram_tensor("dbias", [128, D], f32, kind="ExternalInput").ap()
    out_d = nc.dram_tensor("out", [S, D], f32, kind="ExternalOutput").ap()

    # collective buffers
    g_ag_in = [nc.dram_tensor("g_ag_in0", [192, T], fp16).ap(),
               nc.dram_tensor("g_ag_in1", [192, T], fp16).ap(),
               nc.dram_tensor("g_ag_in2", [384, T], fp16).ap()]
    g_ag_out = [nc.dram_tensor("g_ag_out0", [NCORES * 192, T], fp16,
                               addr_space="Shared").ap(),
                nc.dram_tensor("g_ag_out1", [NCORES * 192, T], fp16,
                               addr_space="Shared").ap(),
                nc.dram_tensor("g_ag_out2", [NCORES * 384, T], fp16,
                               addr_space="Shared").ap()]
    h_a2a_in = nc.dram_tensor("h_a2a_in", [NCORES, 2, HS, S], fp16).ap()
    h_a2a_out = nc.dram_tensor("h_a2a_out", [NCORES, 2, HS, S], fp16).ap()
    y_a2a_in = nc.dram_tensor("y_a2a_in", [NCORES, HS, S], fp16).ap()
    y_a2a_out = nc.dram_tensor("y_a2a_out", [NCORES, HS, S], fp16).ap()

    rg = [list(range(NCORES))]

    with tile.TileContext(nc) as tc, contextlib.ExitStack() as ctx:
        en = tc.nc
        const = ctx.enter_context(tc.tile_pool(name="const", bufs=1))
        xp = ctx.enter_context(tc.tile_pool(name="xres", bufs=1))
        w2p = ctx.enter_context(tc.tile_pool(name="w2p", bufs=6))
        gkp = ctx.enter_context(tc.tile_pool(name="gkp", bufs=3))
        g8p = ctx.enter_context(tc.tile_pool(name="g8p", bufs=6))
        gsp = ctx.enter_context(tc.tile_pool(name="gsp", bufs=2))
        drain = ctx.enter_context(tc.tile_pool(name="drain", bufs=2))
        big = ctx.enter_context(tc.tile_pool(name="big", bufs=1))
        dwp = ctx.enter_context(tc.tile_pool(name="dwp", bufs=N_DW_PRE + 2))
        ps = ctx.enter_context(tc.tile_pool(name="ps", bufs=8, space="PSUM"))

        _cc_prev = [None]

        def cc(kind, ins, outs, waits=()):
            h = en.gpsimd.collective_compute(kind, OP.bypass, ins=ins,
                                             outs=outs, replica_groups=rg)
            for w in waits:
                tile.add_dep_helper(h.ins, w.ins,
                                    reason="collective input writer")
            if _cc_prev[0] is not None:
                tile.add_dep_helper(h.ins, _cc_prev[0].ins,
                                    reason="collective issue-order chain")
            _cc_prev[0] = h
            return h

        # ---------- loads (xh/w1 interleaved per k for earliest start) ----
        xh_s = xp.tile([128, 8, T], fp16, tag="xh")
        w1_s = xp.tile([128, 8, HS], fp16, tag="w1")
        for k in range(8):
            en.sync.dma_start(xh_s[:, k], xh_d[k * 128:(k + 1) * 128])
            en.sync.dma_start(w1_s[:, k], w1h_d[k * 128:(k + 1) * 128])
        x8a_s = xp.tile([128, 4, 2, T], fp8e4, tag="x8a")
        en.sync.dma_start(x8a_s[:], x8a_d.rearrange("a p l t -> p a l t"))
        x8b_s = xp.tile([128, 4, 2, T], fp8e4, tag="x8b")
        en.sync.dma_start(x8b_s[:], x8b_d.rearrange("a p l t -> p a l t"))
        w1a_s = xp.tile([128, 4, 2, HS], fp8e4, tag="w1a")
        en.sync.dma_start(w1a_s[:], w1a_d.rearrange("a p l m -> p a l m"))
        w1b_s = xp.tile([128, 4, 2, HS], fp8e4, tag="w1b")
        en.sync.dma_start(w1b_s[:], w1b_d.rearrange("a p l m -> p a l m"))
        b1_s = const.tile([128, 4], f32, tag="b1")
        en.sync.dma_start(b1_s[:], b1_d.rearrange("m p -> p m"))
        b2y_s = const.tile([128, 4], f32, tag="b2y")
        en.sync.dma_start(b2y_s[:], b2y_d.rearrange("m p -> p m"))
        bc_s = const.tile([128, 4], f32, tag="bc")
        en.sync.dma_start(bc_s[:], bc_d.rearrange("m p -> p m"))
        bd_s = const.tile([128, 4], f32, tag="bd")
        en.sync.dma_start(bd_s[:], bd_d.rearrange("m p -> p m"))
        dbias_s = const.tile([128, D], f32, tag="dbias")
        en.sync.dma_start(dbias_s[:], dbias_d[:])

        N0, N1 = slice(0, 512), slice(512, 1024)

        # ---------- gate1: per m-tile, AG m0 | m1 | m2+m3 ----------
        ag2_wr = []
        for m in range(4):
            mslc = slice(m * 128, (m + 1) * 128)
            p0 = ps.tile([128, 512], f32, tag="ps", name=f"g1_{m}_0")
            p1 = ps.tile([128, 512], f32, tag="ps", name=f"g1_{m}_1")
            for k in range(8):
                w = w1_s[:, k, mslc]
                en.tensor.matmul(p0[:], w, xh_s[:, k, N0],
                                 start=(k == 0), stop=False)
                en.tensor.matmul(p1[:], w, xh_s[:, k, N1],
                                 start=(k == 0), stop=False)
            for a in range(4):
                wa = w1a_s[:, a, :, mslc]
                wb = w1b_s[:, a, :, mslc]
                en.tensor.matmul(p0[:], wa, x8a_s[:, a, :, N0],
                                 start=False, stop=False, perf_mode=DR)
                en.tensor.matmul(p1[:], wa, x8a_s[:, a, :, N1],
                                 start=False, stop=False, perf_mode=DR)
                en.tensor.matmul(p0[:], wb, x8b_s[:, a, :, N0],
                                 start=False, stop=(a == 3), perf_mode=DR)
                en.tensor.matmul(p1[:], wb, x8b_s[:, a, :, N1],
                                 start=False, stop=(a == 3), perf_mode=DR)
            gf = drain.tile([128, T], f32, tag="gf", name=f"gf{m}")
            en.scalar.activation(gf[:, N0], p0[:], AF.Relu,
                                 bias=b1_s[:, m:m + 1], scale=2.0 ** -15)
            en.scalar.activation(gf[:, N1], p1[:], AF.Relu,
                                 bias=b1_s[:, m:m + 1], scale=2.0 ** -15)
            g16 = gsp.tile([128, T], fp16, tag="g16", name=f"g16_{m}")
            en.vector.tensor_copy(g16[:], gf[:])
            glf = drain.tile([128, T], f32, tag="gf", name=f"glf{m}")
            en.vector.tensor_sub(glf[:], gf[:], g16[:])
            gl8 = gsp.tile([128, T], fp8e4, tag="gl8", name=f"gl8_{m}")
            en.vector.tensor_scalar(gl8[:], glf[:], 2.0 ** 12, None,
                                    op0=OP.mult)
            if m < 2:
                wh = en.sync.dma_start(g_ag_in[m][0:128], g16[:])
                lo_dst = g_ag_in[m][128:192] \
                    .bitcast(fp8e4).rearrange("a (b f) -> (a b) f", b=2)
                wl = en.sync.dma_start(lo_dst, gl8[:])
                cc("AllGather", [g_ag_in[m][:]], [g_ag_out[m][:]],
                   waits=(wh, wl))
            else:
                r = (m - 2) * 128
                wh = en.sync.dma_start(g_ag_in[2][r:r + 128], g16[:])
                lo_dst = g_ag_in[2][256 + (m - 2) * 64: 256 + (m - 1) * 64] \
                    .bitcast(fp8e4).rearrange("a (b f) -> (a b) f", b=2)
                wl = en.sync.dma_start(lo_dst, gl8[:])
                ag2_wr += [wh, wl]
                if m == 3:
                    cc("AllGather", [g_ag_in[2][:]], [g_ag_out[2][:]],
                       waits=tuple(ag2_wr))

        # ---------- up projections (fp16, combined a2a) ----------
        up_s = {}
        for nm, src in (("c", upc_d), ("d", upd_d)):
            u = xp.tile([128, 8, HS], fp16, tag=f"up{nm}")
            en.sync.dma_start(u[:], src.rearrange("(ko p) m -> p ko m", p=128))
            up_s[nm] = u
        h_wr = []
        h_in_v = h_a2a_in.rearrange("j b (p a2) s -> p b a2 j s", a2=4)
        for bi, (nm, bias_t) in enumerate((("c", bc_s), ("d", bd_s))):
            for m in range(4):
                mslc = slice(m * 128, (m + 1) * 128)
                p0 = ps.tile([128, 512], f32, tag="ps", name=f"u{nm}_{m}_0")
                p1 = ps.tile([128, 512], f32, tag="ps", name=f"u{nm}_{m}_1")
                for k in range(8):
                    w = up_s[nm][:, k, mslc]
                    en.tensor.matmul(p0[:], w, xh_s[:, k, N0],
                                     start=(k == 0), stop=(k == 7))
                    en.tensor.matmul(p1[:], w, xh_s[:, k, N1],
                                     start=(k == 0), stop=(k == 7))
                hq = drain.tile([128, T], fp16, tag="hq", name=f"hq{nm}{m}")
                en.scalar.activation(hq[:, N0], p0[:], AF.Identity,
                                     bias=bias_t[:, m:m + 1])
                en.scalar.activation(hq[:, N1], p1[:], AF.Identity,
                                     bias=bias_t[:, m:m + 1])
                h_wr.append(en.sync.dma_start(h_in_v[:, bi, m], hq[:]))
        h_cc = cc("AllToAll", [h_a2a_in[:]], [h_a2a_out[:]],
                  waits=tuple(h_wr))

        # ---------- gate2: k-tiles in AG order ----------
        pts = {(m, n): ps.tile([128, 512], f32, tag="ps", name=f"g2_{m}_{n}")
               for m in range(4) for n in range(2)}
        n_dw = [0]
        dw_tiles = {}

        def issue_dw(n=1):
            for _ in range(n):
                i = n_dw[0]
                if i >= KT:
                    return
                n_dw[0] += 1
                dwk = dwp.tile([128, D], fp16, tag="dwk", name=f"dwk{i}")
                en.sync.dma_start(dwk[:], dw_d[i * 128:(i + 1) * 128])
                dw_tiles[i] = dwk

        for a in range(KT // 2):
            g8ap = g8p.tile([128, 2, T], fp8e4, tag="g8a", name=f"g8a{a}")
            g8bp = g8p.tile([128, 2, T], fp8e4, tag="g8b", name=f"g8b{a}")
            gk2 = gkp.tile([128, 2, T], fp16, tag="gk", name=f"gk{a}")
            for half in range(2):
                idx = 2 * a + half
                kt = KT_ORDER[idx]
                j, hi_row, lo_row = _ag_pos(kt)
                en.sync.dma_start(gk2[:, half],
                                  g_ag_out[j][hi_row:hi_row + 128])
                lo_src = g_ag_out[j][lo_row:lo_row + 64] \
                    .bitcast(fp8e4).rearrange("a (b f) -> (a b) f", b=2)
                en.sync.dma_start(g8bp[:, half], lo_src)
                en.scalar.activation(g8ap[:, half], gk2[:, half],
                                     AF.Identity, scale=0.5)
                w2k = w2p.tile([128, HS], fp16, tag="w2k", name=f"w2k{idx}")
                en.sync.dma_start(w2k[:], w2h_d[idx])
                first = (idx == 0)
                for m in range(4):
                    mslc = slice(m * 128, (m + 1) * 128)
                    en.tensor.matmul(pts[(m, 0)][:], w2k[:, mslc],
                                     gk2[:, half, N0], start=first,
                                     stop=False)
                    en.tensor.matmul(pts[(m, 1)][:], w2k[:, mslc],
                                     gk2[:, half, N1], start=first,
                                     stop=False)
            w2ak = w2p.tile([128, 2, HS], fp8e4, tag="w2c", name=f"w2a{a}")
            en.sync.dma_start(w2ak[:], w2a_d[a])
            w2bk = w2p.tile([128, 2, HS], fp8e4, tag="w2c", name=f"w2b{a}")
            en.sync.dma_start(w2bk[:], w2b_d[a])
            last = (a == KT // 2 - 1)
            for m in range(4):
                mslc = slice(m * 128, (m + 1) * 128)
                en.tensor.matmul(pts[(m, 0)][:], w2ak[:, :, mslc],
                                 g8ap[:, :, N0], start=False, stop=False,
                                 perf_mode=DR)
                en.tensor.matmul(pts[(m, 1)][:], w2ak[:, :, mslc],
                                 g8ap[:, :, N1], start=False, stop=False,
                                 perf_mode=DR)
                en.tensor.matmul(pts[(m, 0)][:], w2bk[:, :, mslc],
                                 g8bp[:, :, N0], start=False, stop=last,
                                 perf_mode=DR)
                en.tensor.matmul(pts[(m, 1)][:], w2bk[:, :, mslc],
                                 g8bp[:, :, N1], start=False, stop=last,
                                 perf_mode=DR)
            if a >= 2 and a % 2 == 0:
                issue_dw(2)

        # drains straight to y16 = (score - Y_OFF)*Y_SCL in fp16.
        # a2a block rows are (p, a2)-ordered so the receive side fills with
        # one contiguous-per-partition DMA.
        y_wr = []
        y_in_v = y_a2a_in.rearrange("j (p a2) s -> p a2 j s", a2=4)
        for m in range(4):
            ym = drain.tile([128, T], fp16, tag="hq", name=f"ym{m}")
            en.scalar.activation(ym[:, N0], pts[(m, 0)][:], AF.Identity,
                                 bias=b2y_s[:, m:m + 1], scale=2.0 ** -9)
            en.scalar.activation(ym[:, N1], pts[(m, 1)][:], AF.Identity,
                                 bias=b2y_s[:, m:m + 1], scale=2.0 ** -9)
            y_wr.append(en.sync.dma_start(y_in_v[:, m], ym[:]))
        y_cc = cc("AllToAll", [y_a2a_in[:]], [y_a2a_out[:]],
                  waits=tuple(y_wr))
        issue_dw(N_DW_PRE - n_dw[0])

        # ---------- h fills (wait h a2a) + gelu precompute ----------
        hc_s = big.tile([128, KT, S], fp16, tag="hc", name="hc_s")
        hd_s = big.tile([128, KT, S], fp16, tag="hd", name="hd_s")
        gp_s = big.tile([128, KT, S], fp16, tag="gp", name="gp_s")
        h_src = h_a2a_out.rearrange("r b (p a2) s -> p b r (a2 s)", a2=4)
        for bi, dst in ((0, hc_s), (1, hd_s)):
            hr = en.sync.dma_start(
                dst[:].rearrange("p (r a2) s -> p r (a2 s)", a2=4),
                h_src[:, bi])
            tile.add_dep_helper(hr.ins, h_cc.ins, reason="h a2a done")
        # gp := hc + hd; gc := gelu(hc) -> hd_s; gelu(gp) -> hc_s;
        # gd := gelu(hp) - gc -> gp_s.  (runs during the y a2a wait)
        en.vector.tensor_tensor(gp_s[:], hc_s[:], hd_s[:], op=OP.add)
        en.scalar.activation(hd_s[:], hc_s[:], AF.Gelu)
        en.scalar.activation(hc_s[:], gp_s[:], AF.Gelu)
        en.vector.tensor_sub(gp_s[:], hc_s[:], hd_s[:])
        a_s = hc_s        # gelu(hp) content is dead once gd is computed

        # PE warm-keeper: a short matmul burst after gate2 drains plus a
        # couple per search round keeps the HAM clock gate open so the
        # down projection runs at full rate (outputs unused)
        pwm = ps.tile([128, 512], f32, tag="ps", name="pwm")

        def warm_mms(n):
            for _ in range(n):
                en.tensor.matmul(pwm[:], up_s["c"][:, 0, 0:128],
                                 up_s["c"][:, 0, :], start=True, stop=True)

        warm_mms(8)

        # ---------- y16 fill (two parallel DMA queues) ----------
        y16 = big.tile([128, KT, S], fp16, tag="y16", name="y16")
        yfills = []
        for hh, eng in ((0, en.sync), (1, en.scalar)):
            yf = eng.dma_start(
                y16[:, hh * 16:(hh + 1) * 16, :]
                .rearrange("p (r a2) s -> p r (a2 s)", a2=4),
                y_a2a_out[hh * 4:(hh + 1) * 4]
                .rearrange("r (p a2) s -> p r (a2 s)", a2=4))
            tile.add_dep_helper(yf.ins, y_cc.ins, reason="y a2a done")
            yfills.append(yf)

        # ---------- Newton threshold search on y16 ----------
        y_flat = y16.rearrange("p a b -> p (a b)")
        half = (KT * S) // 2
        # scratch targets for the count passes: carve them out of the
        # long-dead xh tile (count writes are garbage, only accum matters)
        cmpb = xh_s[:, 0:1, :].bitcast(fp8e4).rearrange("p a t -> p (a t)")
        sgnb = xh_s[:, 2:3, :].bitcast(fp8e4).rearrange("p a t -> p (a t)")
        ones32 = const.tile([128, 128], f32, tag="ones32", name="ones32")
        en.vector.memset(ones32[:], 1.0)
        yt = const.tile([128, 1], f32, tag="yt", name="yt")
        nyt = const.tile([128, 1], f32, tag="nyt", name="nyt")
        uu = const.tile([128, 1], f32, tag="uu", name="uu")
        en.vector.memset(yt[:], 0.0)
        en.vector.memset(nyt[:], 0.0)
        accs = const.tile([128, 2], f32, tag="accs", name="accs")
        rmax = const.tile([128, KT], f32, tag="rmax", name="rmax")

        for r in range(R_ITER):
            hv = en.vector.tensor_scalar(cmpb, y_flat[:, 0:half],
                                         yt[:], 0.0, op0=OP.is_gt,
                                         op1=OP.add, accum_out=accs[:, 0:1])
            hs = en.scalar.activation(sgnb, y_flat[:, half:],
                                      AF.Sign, bias=nyt[:],
                                      accum_out=accs[:, 1:2])
            if r == 0:
                for dep in yfills:
                    tile.add_dep_helper(hv.ins, dep.ins, reason="y16 ready")
                    tile.add_dep_helper(hs.ins, dep.ins, reason="y16 ready")
            en.vector.scalar_tensor_tensor(uu[:], accs[:, 1:2], 0.5,
                                           accs[:, 0:1],
                                           op0=OP.mult, op1=OP.add)
            en.vector.tensor_scalar(uu[:], uu[:], 768.0, None, op0=OP.add)
            pred = ps.tile([128, 1], f32, tag="ps", name=f"pred{r}")
            en.tensor.matmul(pred[:], ones32[:], uu[:],
                             start=True, stop=True)
            yt_h = en.vector.scalar_tensor_tensor(yt[:], pred[:], C_NEWTON,
                                                  yt[:],
                                                  op0=OP.mult, op1=OP.add)
            if r == 0:
                round0_yt = yt_h
            en.vector.tensor_scalar(nyt[:], yt[:], -1.0, None, op0=OP.mult)
            warm_mms(3)
            if r == 1:
                # rowmax (needed only for the final select) slots into
                # vector gaps between rounds; pin it after round 0 so the
                # scheduler cannot hoist it ahead of the first count
                for q in range(4):
                    hq_ = en.vector.reduce_max(rmax[:, q * 8:(q + 1) * 8],
                                               y16[:, q * 8:(q + 1) * 8, :],
                                               axis=mybir.AxisListType.X)
                    tile.add_dep_helper(hq_.ins, round0_yt.ins,
                                        reason="defer rowmax past round 0")

        sel = const.tile([128, KT], f32, tag="sel", name="sel")
        en.vector.tensor_scalar(sel[:], rmax[:], yt[:], None, op0=OP.is_gt)

        # ---------- select + down, pipelined per 8-ktile chunk ----------
        # a := gd*sel + gc  (gd in gp_s, gc in hd_s) -> a_s in hc_s
        pd0 = ps.tile([128, 512], f32, tag="ps", name="dn0")
        pd1 = ps.tile([128, 512], f32, tag="ps", name="dn1")
        for q in range(4):
            issue_dw(8)
            for kt in range(q * 8, (q + 1) * 8):
                en.vector.scalar_tensor_tensor(
                    a_s[:, kt, :], gp_s[:, kt, :], sel[:, kt:kt + 1],
                    hd_s[:, kt, :], op0=OP.mult, op1=OP.add)
            for kt in range(q * 8, (q + 1) * 8):
                dwk = dw_tiles[kt]
                en.tensor.matmul(pd0[:], a_s[:, kt, :], dwk[:, N0],
                                 start=(kt == 0), stop=(kt == KT - 1))
                en.tensor.matmul(pd1[:], a_s[:, kt, :], dwk[:, N1],
                                 start=(kt == 0), stop=(kt == KT - 1))
        osb = drain.tile([128, D], f32, tag="gf", name="osb")
        en.vector.tensor_tensor(osb[:, N0], pd0[:], dbias_s[:, N0], op=OP.add)
        en.sync.dma_start(out_d[:, N0], osb[:, N0])
        en.vector.tensor_tensor(osb[:, N1], pd1[:], dbias_s[:, N1], op=OP.add)
        en.sync.dma_start(out_d[:, N1], osb[:, N1])

    nc.compile()
    return nc


_NC_CACHE = None


def _f16hi(a, scale):
    """fp16(a*scale) and the fp32 residual a - fp16(a*scale)/scale."""
    hi = (a.astype(np.float64) * scale).astype(F16)
    res = (a.astype(np.float64) - hi.astype(np.float64) / scale).astype(F32)
    return hi, res


def _pair_k(a):
    """[n*256, X] -> [n, 128, 2, X] : (2t, 2t+1) k-tile pairs."""
    n = a.shape[0] // 256
    return np.ascontiguousarray(a.reshape(n, 2, 128, -1).transpose(0, 2, 1, 3))


def _prep_in_maps(x, w1, b1, w2, b2, upw, upb, ucw, ucb, dw, db):
    xt = np.ascontiguousarray(x.reshape(T, D).T).astype(F32)   # [D, T]
    xh16 = xt.astype(F16)
    x_lo = xt - xh16.astype(F32)
    x8a = _pair_k((xt * 0.5).astype(FP8))
    x8b = _pair_k((x_lo * (2.0 ** 9)).astype(FP8))
    udw = upw - ucw
    udb = upb - ucb
    dwT16 = np.ascontiguousarray(dw.T.astype(F16))             # [H, D]
    dbias = np.ascontiguousarray(np.tile(db[None, :], (128, 1)).astype(F32))

    perm = (np.asarray(KT_ORDER)[:, None] * 128
            + np.arange(128)[None, :]).reshape(-1)

    in_maps = []
    for c in range(NCORES):
        sh = slice(c * HS, (c + 1) * HS)
        w1t = np.ascontiguousarray(w1[sh].T).astype(F32)       # [D, HS]
        w1h, w1res = _f16hi(w1t, 2.0 ** 15)
        w2t = np.ascontiguousarray(w2[sh].T).astype(F32)       # [H, HS]
        w2t = np.ascontiguousarray(w2t[perm])
        w2h, w2res = _f16hi(w2t, 2.0 ** 15)
        b2y = (b2[sh].astype(F32) - Y_OFF) * Y_SCL
        in_maps.append({
            "xh16": xh16,
            "x8a": x8a,
            "x8b": x8b,
            "w1h16": w1h,
            "w1a8": _pair_k((w1res * (2.0 ** 16)).astype(FP8)),
            "w1b8": _pair_k((w1t * (2.0 ** 6)).astype(FP8)),
            "w2h16": np.ascontiguousarray(w2h.reshape(KT, 128, HS)),
            "w2a8": _pair_k((w2res * (2.0 ** 16)).astype(FP8)),
            "w2b8": _pair_k((w2t * (2.0 ** 3)).astype(FP8)),
            "upc16": np.ascontiguousarray(ucw[sh].T.astype(F16)),
            "upd16": np.ascontiguousarray(udw[sh].T.astype(F16)),
            "dwT16": dwT16,
            "b1s": np.ascontiguousarray(b1[sh].reshape(4, 128)).astype(F32),
            "b2ys": np.ascontiguousarray(b2y.reshape(4, 128)).astype(F32),
            "bcs": np.ascontiguousarray(ucb[sh].reshape(4, 128)).astype(F32),
            "bds": np.ascontiguousarray(udb[sh].reshape(4, 128)).astype(F32),
            "dbias": dbias,
        })
    return in_maps


def kernel_in_maps(**inputs):
    names = ["inputs", "gate_w1", "gate_b1", "gate_w2", "gate_b2",
             "up_prev_w", "up_prev_b", "up_curr_w", "up_curr_b",
             "down_w", "down_b"]
    vals = [np.asarray(inputs[n], F32) for n in names]
    return _prep_in_maps(*vals)


def kernel(**inputs):
    global _NC_CACHE
    if _NC_CACHE is None:
        _NC_CACHE = _build()
    nc = _NC_CACHE
    in_maps = kernel_in_maps(**inputs)
    res = run_bass_kernel_spmd(nc, in_maps, core_ids=list(range(NCORES)))
    out = np.stack([res.results[c]["out"] for c in range(NCORES)], axis=0)
    return np.ascontiguousarray(out.astype(F32))


# revision 35
# speedup vs baseline: 1.1249x; 1.0018x over previous
"""Trainium2 Bass kernel for nn_PraxisScatter (moe_routing) — v5.

Strategy (8 NeuronCores):
  - gate1 tensor-parallel over H (512 rows/core), 3-term fp16-hi + fp8
    cross corrections at PSUM scale 2^15; drains fp32 g.
  - g AllGathered in 3 packed chunks (m0 | m1 | m2+m3), each ONE
    collective carrying fp16 hi + bit-packed fp8 lo-residual.  The first
    chunk rides the cross-core rendezvous.
  - gate2 tensor-parallel 3-term fp16-hi + fp8 DR crosses, k-tiles in
    AG-chunk order; w2 host-permuted to match.  PSUM drains straight to
    y16 = (score-0.361)*64 fp16 (bias folded), so the score exchange is
    a 1MB fp16 AllToAll and needs no receive-side conversion.
  - up projections fp16 single-term; hc+hd exchanged in ONE combined
    AllToAll (fp16); gelu(hc) and gelu(hc+hd) precomputed during the
    score-exchange wait so the post-threshold tail is select+down only.
  - threshold via fixed-slope Newton on exact fp16 counts (vector+scalar
    halves) with a fp32 ones-matmul partition reduce+broadcast; PE
    re-warm burst during the search keeps the down matmuls at full clock.
  - fp16 down projection; weights prefetched during gate2/search.
"""

import sys

try:
    import concourse  # noqa: F401
except ImportError:  # pragma: no cover
    sys.path.insert(0, "/opt/trn_rl_repo")

import contextlib

import ml_dtypes
import numpy as np

import concourse.bass as bass  # noqa: F401
import concourse.mybir as mybir
import concourse.tile as tile
from concourse import bacc
from concourse.bass_utils import run_bass_kernel_spmd

BF16 = ml_dtypes.bfloat16
F16 = np.float16
F32 = np.float32
FP8 = ml_dtypes.float8_e4m3

NCORES = 8
B, S, D, H = 8, 128, 1024, 4096
T = B * S              # 1024 tokens
HS = H // NCORES       # 512 h rows per core
KT = H // 128          # 32 k-tiles over the full H
K_SEL = 256 * S        # 32768
Y_OFF, Y_SCL = 0.361, 64.0
C_NEWTON = 1.0 / 4260.0
R_ITER = 3
N_DW_PRE = 16          # dw tiles prefetched during gate2/search

f32 = mybir.dt.float32
bf16 = mybir.dt.bfloat16
fp16 = mybir.dt.float16
fp8e4 = mybir.dt.float8e4
AF = mybir.ActivationFunctionType
OP = mybir.AluOpType
DR = mybir.MatmulPerfMode.DoubleRow

# gate2 k-tile order (same on every core): AG chunk0 (every core's m0),
# chunk1 (m1), chunk2 (m2+m3).  k-tile kt covers global h rows kt*128..
KT_ORDER = ([4 * c for c in range(NCORES)]
            + [4 * c + 1 for c in range(NCORES)]
            + [4 * c + i for c in range(NCORES) for i in (2, 3)])


def _ag_pos(kt):
    """(chunk j, hi-row, lo-row) of k-tile kt inside g_ag_out[j]."""
    c, i = kt // 4, kt % 4
    if i < 2:
        return i, c * 192, c * 192 + 128
    return 2, c * 384 + (i - 2) * 128, c * 384 + 256 + (i - 2) * 64


def _build():
    nc = bacc.Bacc("TRN2", target_bir_lowering=False, debug=False,
                   num_devices=NCORES)

    xh_d = nc.dram_tensor("xh16", [D, T], fp16, kind="ExternalInput").ap()
    x8a_d = nc.dram_tensor("x8a", [4, 128, 2, T], fp8e4, kind="ExternalInput").ap()
    x8b_d = nc.dram_tensor("x8b", [4, 128, 2, T], fp8e4, kind="ExternalInput").ap()
    w1h_d = nc.dram_tensor("w1h16", [D, HS], fp16, kind="ExternalInput").ap()
    w1a_d = nc.dram_tensor("w1a8", [4, 128, 2, HS], fp8e4, kind="ExternalInput").ap()
    w1b_d = nc.dram_tensor("w1b8", [4, 128, 2, HS], fp8e4, kind="ExternalInput").ap()
    w2h_d = nc.dram_tensor("w2h16", [KT, 128, HS], fp16, kind="ExternalInput").ap()
    w2a_d = nc.dram_tensor("w2a8", [KT // 2, 128, 2, HS], fp8e4, kind="ExternalInput").ap()
    w2b_d = nc.dram_tensor("w2b8", [KT // 2, 128, 2, HS], fp8e4, kind="ExternalInput").ap()
    upc_d = nc.dram_tensor("upc16", [D, HS], fp16, kind="ExternalInput").ap()
    upd_d = nc.dram_tensor("upd16", [D, HS], fp16, kind="ExternalInput").ap()
    dw_d = nc.dram_tensor("dwT16", [H, D], fp16, kind="ExternalInput").ap()
    b1_d = nc.dram_tensor("b1s", [4, 128], f32, kind="ExternalInput").ap()
    b2y_d = nc.dram_tensor("b2ys", [4, 128], f32, kind="ExternalInput").ap()
    bc_d = nc.dram_tensor("bcs", [4, 128], f32, kind="ExternalInput").ap()
    bd_d = nc.dram_tensor("bds", [4, 128], f32, kind="ExternalInput").ap()
    dbias_d = nc.dram_tensor("dbias", [128, D], f32, kind="ExternalInput").ap()
    out_d = nc.dram_tensor("out", [S, D], f32, kind="ExternalOutput").ap()

    # collective buffers
    g_ag_in = [nc.dram_tensor("g_ag_in0", [192, T], fp16).ap(),
               nc.dram_tensor("g_ag_in1", [192, T], fp16).ap(),
               nc.dram_tensor("g_ag_in2", [384, T], fp16).ap()]
    g_ag_out = [nc.dram_tensor("g_ag_out0", [NCORES * 192, T], fp16,
                               addr_space="Shared").ap(),
                nc.dram_tensor("g_ag_out1", [NCORES * 192, T], fp16,
                               addr_space="Shared").ap(),
                nc.dram_tensor("g_ag_out2", [NCORES * 384, T], fp16,
                               addr_space="Shared").ap()]
    h_a2a_in = nc.dram_tensor("h_a2a_in", [NCORES, 2, HS, S], fp16).ap()
    h_a2a_out = nc.dram_tensor("h_a2a_out", [NCORES, 2, HS, S], fp16).ap()
    y_a2a_in = nc.dram_tensor("y_a2a_in", [NCORES, HS, S], fp16).ap()
    y_a2a_out = nc.dram_tensor("y_a2a_out", [NCORES, HS, S], fp16).ap()

    rg = [list(range(NCORES))]

    with tile.TileContext(nc) as tc, contextlib.ExitStack() as ctx:
        en = tc.nc
        const = ctx.enter_context(tc.tile_pool(name="const", bufs=1))
        xp = ctx.enter_context(tc.tile_pool(name="xres", bufs=1))
        w2p = ctx.enter_context(tc.tile_pool(name="w2p", bufs=6))
        gkp = ctx.enter_context(tc.tile_pool(name="gkp", bufs=3))
        g8p = ctx.enter_context(tc.tile_pool(name="g8p", bufs=6))
        gsp = ctx.enter_context(tc.tile_pool(name="gsp", bufs=2))
        drain = ctx.enter_context(tc.tile_pool(name="drain", bufs=2))
        big = ctx.enter_context(tc.tile_pool(name="big", bufs=1))
        dwp = ctx.enter_context(tc.tile_pool(name="dwp", bufs=N_DW_PRE + 2))
        ps = ctx.enter_context(tc.tile_pool(name="ps", bufs=8, space="PSUM"))

        _cc_prev = [None]

        def cc(kind, ins, outs, waits=()):
            h = en.gpsimd.collective_compute(kind, OP.bypass, ins=ins,
                                             outs=outs, replica_groups=rg)
            for w in waits:
                tile.add_dep_helper(h.ins, w.ins,
                                    reason="collective input writer")
            if _cc_prev[0] is not None:
                tile.add_dep_helper(h.ins, _cc_prev[0].ins,
                                    reason="collective issue-order chain")
            _cc_prev[0] = h
            return h

        # ---------- loads (xh/w1 interleaved per k for earliest start) ----
        xh_s = xp.tile([128, 8, T], fp16, tag="xh")
        w1_s = xp.tile([128, 8, HS], fp16, tag="w1")
        for k in range(8):
            en.sync.dma_start(xh_s[:, k], xh_d[k * 128:(k + 1) * 128])
            en.sync.dma_start(w1_s[:, k], w1h_d[k * 128:(k + 1) * 128])
        x8a_s = xp.tile([128, 4, 2, T], fp8e4, tag="x8a")
        en.sync.dma_start(x8a_s[:], x8a_d.rearrange("a p l t -> p a l t"))
        x8b_s = xp.tile([128, 4, 2, T], fp8e4, tag="x8b")
        en.sync.dma_start(x8b_s[:], x8b_d.rearrange("a p l t -> p a l t"))
        w1a_s = xp.tile([128, 4, 2, HS], fp8e4, tag="w1a")
        en.sync.dma_start(w1a_s[:], w1a_d.rearrange("a p l m -> p a l m"))
        w1b_s = xp.tile([128, 4, 2, HS], fp8e4, tag="w1b")
        en.sync.dma_start(w1b_s[:], w1b_d.rearrange("a p l m -> p a l m"))
        b1_s = const.tile([128, 4], f32, tag="b1")
        en.sync.dma_start(b1_s[:], b1_d.rearrange("m p -> p m"))
        b2y_s = const.tile([128, 4], f32, tag="b2y")
        en.sync.dma_start(b2y_s[:], b2y_d.rearrange("m p -> p m"))
        bc_s = const.tile([128, 4], f32, tag="bc")
        en.sync.dma_start(bc_s[:], bc_d.rearrange("m p -> p m"))
        bd_s = const.tile([128, 4], f32, tag="bd")
        en.sync.dma_start(bd_s[:], bd_d.rearrange("m p -> p m"))
        dbias_s = const.tile([128, D], f32, tag="dbias")
        en.sync.dma_start(dbias_s[:], dbias_d[:])

        N0, N1 = slice(0, 512), slice(512, 1024)

        # ---------- gate1: per m-tile, AG m0 | m1 | m2+m3 ----------
        ag2_wr = []
        for m in range(4):
            mslc = slice(m * 128, (m + 1) * 128)
            p0 = ps.tile([128, 512], f32, tag="ps", name=f"g1_{m}_0")
            p1 = ps.tile([128, 512], f32, tag="ps", name=f"g1_{m}_1")
            for k in range(8):
                w = w1_s[:, k, mslc]
                en.tensor.matmul(p0[:], w, xh_s[:, k, N0],
                                 start=(k == 0), stop=False)
                en.tensor.matmul(p1[:], w, xh_s[:, k, N1],
                                 start=(k == 0), stop=False)
            for a in range(4):
                wa = w1a_s[:, a, :, mslc]
                wb = w1b_s[:, a, :, mslc]
                en.tensor.matmul(p0[:], wa, x8a_s[:, a, :, N0],
                                 start=False, stop=False, perf_mode=DR)
                en.tensor.matmul(p1[:], wa, x8a_s[:, a, :, N1],
                                 start=False, stop=False, perf_mode=DR)
                en.tensor.matmul(p0[:], wb, x8b_s[:, a, :, N0],
                                 start=False, stop=(a == 3), perf_mode=DR)
                en.tensor.matmul(p1[:], wb, x8b_s[:, a, :, N1],
                                 start=False, stop=(a == 3), perf_mode=DR)
            gf = drain.tile([128, T], f32, tag="gf", name=f"gf{m}")
            en.scalar.activation(gf[:, N0], p0[:], AF.Relu,
                                 bias=b1_s[:, m:m + 1], scale=2.0 ** -15)
            en.scalar.activation(gf[:, N1], p1[:], AF.Relu,
                                 bias=b1_s[:, m:m + 1], scale=2.0 ** -15)
            g16 = gsp.tile([128, T], fp16, tag="g16", name=f"g16_{m}")
            en.vector.tensor_copy(g16[:], gf[:])
            glf = drain.tile([128, T], f32, tag="gf", name=f"glf{m}")
            en.vector.tensor_sub(glf[:], gf[:], g16[:])
            gl8 = gsp.tile([128, T], fp8e4, tag="gl8", name=f"gl8_{m}")
            en.vector.tensor_scalar(gl8[:], glf[:], 2.0 ** 12, None,
                                    op0=OP.mult)
            if m < 2:
                wh = en.sync.dma_start(g_ag_in[m][0:128], g16[:])
                lo_dst = g_ag_in[m][128:192] \
                    .bitcast(fp8e4).rearrange("a (b f) -> (a b) f", b=2)
                wl = en.sync.dma_start(lo_dst, gl8[:])
                cc("AllGather", [g_ag_in[m][:]], [g_ag_out[m][:]],
                   waits=(wh, wl))
            else:
                r = (m - 2) * 128
                wh = en.sync.dma_start(g_ag_in[2][r:r + 128], g16[:])
                lo_dst = g_ag_in[2][256 + (m - 2) * 64: 256 + (m - 1) * 64] \
                    .bitcast(fp8e4).rearrange("a (b f) -> (a b) f", b=2)
                wl = en.sync.dma_start(lo_dst, gl8[:])
                ag2_wr += [wh, wl]
                if m == 3:
                    cc("AllGather", [g_ag_in[2][:]], [g_ag_out[2][:]],
                       waits=tuple(ag2_wr))

        # ---------- up projections (fp16, combined a2a) ----------
        up_s = {}
        for nm, src in (("c", upc_d), ("d", upd_d)):
            u = xp.tile([128, 8, HS], fp16, tag=f"up{nm}")
            en.sync.dma_start(u[:], src.rearrange("(ko p) m -> p ko m", p=128))
            up_s[nm] = u
        h_wr = []
        h_in_v = h_a2a_in.rearrange("j b (p a2) s -> p b a2 j s", a2=4)
        for bi, (nm, bias_t) in enumerate((("c", bc_s), ("d", bd_s))):
            for m in range(4):
                mslc = slice(m * 128, (m + 1) * 128)
                p0 = ps.tile([128, 512], f32, tag="ps", name=f"u{nm}_{m}_0")
                p1 = ps.tile([128, 512], f32, tag="ps", name=f"u{nm}_{m}_1")
                for k in range(8):
                    w = up_s[nm][:, k, mslc]
                    en.tensor.matmul(p0[:], w, xh_s[:, k, N0],
                                     start=(k == 0), stop=(k == 7))
                    en.tensor.matmul(p1[:], w, xh_s[:, k, N1],
                                     start=(k == 0), stop=(k == 7))
                hq = drain.tile([128, T], fp16, tag="hq", name=f"hq{nm}{m}")
                en.scalar.activation(hq[:, N0], p0[:], AF.Identity,
                                     bias=bias_t[:, m:m + 1])
                en.scalar.activation(hq[:, N1], p1[:], AF.Identity,
                                     bias=bias_t[:, m:m + 1])
                h_wr.append(en.sync.dma_start(h_in_v[:, bi, m], hq[:]))
        h_cc = cc("AllToAll", [h_a2a_in[:]], [h_a2a_out[:]],
                  waits=tuple(h_wr))

        # ---------- gate2: k-tiles in AG order ----------
        pts = {(m, n): ps.tile([128, 512], f32, tag="ps", name=f"g2_{m}_{n}")
               for m in range(4) for n in range(2)}
        n_dw = [0]
        dw_tiles = {}

        def issue_dw(n=1):
            for _ in range(n):
                i = n_dw[0]
                if i >= KT:
                    return
                n_dw[0] += 1
                dwk = dwp.tile([128, D], fp16, tag="dwk", name=f"dwk{i}")
                en.sync.dma_start(dwk[:], dw_d[i * 128:(i + 1) * 128])
                dw_tiles[i] = dwk

        for a in range(KT // 2):
            g8ap = g8p.tile([128, 2, T], fp8e4, tag="g8a", name=f"g8a{a}")
            g8bp = g8p.tile([128, 2, T], fp8e4, tag="g8b", name=f"g8b{a}")
            gk2 = gkp.tile([128, 2, T], fp16, tag="gk", name=f"gk{a}")
            for half in range(2):
                idx = 2 * a + half
                kt = KT_ORDER[idx]
                j, hi_row, lo_row = _ag_pos(kt)
                en.sync.dma_start(gk2[:, half],
                                  g_ag_out[j][hi_row:hi_row + 128])
                lo_src = g_ag_out[j][lo_row:lo_row + 64] \
                    .bitcast(fp8e4).rearrange("a (b f) -> (a b) f", b=2)
                en.sync.dma_start(g8bp[:, half], lo_src)
                en.scalar.activation(g8ap[:, half], gk2[:, half],
                                     AF.Identity, scale=0.5)
                w2k = w2p.tile([128, HS], fp16, tag="w2k", name=f"w2k{idx}")
                en.scalar.dma_start(w2k[:], w2h_d[idx])
                first = (idx == 0)
                for m in range(4):
                    mslc = slice(m * 128, (m + 1) * 128)
                    en.tensor.matmul(pts[(m, 0)][:], w2k[:, mslc],
                                     gk2[:, half, N0], start=first,
                                     stop=False)
                    en.tensor.matmul(pts[(m, 1)][:], w2k[:, mslc],
                                     gk2[:, half, N1], start=first,
                                     stop=False)
            w2ak = w2p.tile([128, 2, HS], fp8e4, tag="w2c", name=f"w2a{a}")
            en.scalar.dma_start(w2ak[:], w2a_d[a])
            w2bk = w2p.tile([128, 2, HS], fp8e4, tag="w2c", name=f"w2b{a}")
            en.scalar.dma_start(w2bk[:], w2b_d[a])
            last = (a == KT // 2 - 1)
            for m in range(4):
                mslc = slice(m * 128, (m + 1) * 128)
                en.tensor.matmul(pts[(m, 0)][:], w2ak[:, :, mslc],
                                 g8ap[:, :, N0], start=False, stop=False,
                                 perf_mode=DR)
                en.tensor.matmul(pts[(m, 1)][:], w2ak[:, :, mslc],
                                 g8ap[:, :, N1], start=False, stop=False,
                                 perf_mode=DR)
                en.tensor.matmul(pts[(m, 0)][:], w2bk[:, :, mslc],
                                 g8bp[:, :, N0], start=False, stop=last,
                                 perf_mode=DR)
                en.tensor.matmul(pts[(m, 1)][:], w2bk[:, :, mslc],
                                 g8bp[:, :, N1], start=False, stop=last,
                                 perf_mode=DR)
            if a >= 2 and a % 2 == 0:
                issue_dw(2)

        # drains straight to y16 = (score - Y_OFF)*Y_SCL in fp16.
        # a2a block rows are (p, a2)-ordered so the receive side fills with
        # one contiguous-per-partition DMA.
        y_wr = []
        y_in_v = y_a2a_in.rearrange("j (p a2) s -> p a2 j s", a2=4)
        for m in range(4):
            ym = drain.tile([128, T], fp16, tag="hq", name=f"ym{m}")
            en.scalar.activation(ym[:, N0], pts[(m, 0)][:], AF.Identity,
                                 bias=b2y_s[:, m:m + 1], scale=2.0 ** -9)
            en.scalar.activation(ym[:, N1], pts[(m, 1)][:], AF.Identity,
                                 bias=b2y_s[:, m:m + 1], scale=2.0 ** -9)
            y_wr.append(en.sync.dma_start(y_in_v[:, m], ym[:]))
        y_cc = cc("AllToAll", [y_a2a_in[:]], [y_a2a_out[:]],
                  waits=tuple(y_wr))
        issue_dw(N_DW_PRE - n_dw[0])

        # ---------- h fills (wait h a2a) + gelu precompute ----------
        hc_s = big.tile([128, KT, S], fp16, tag="hc", name="hc_s")
        hd_s = big.tile([128, KT, S], fp16, tag="hd", name="hd_s")
        gp_s = big.tile([128, KT, S], fp16, tag="gp", name="gp_s")
        h_src = h_a2a_out.rearrange("r b (p a2) s -> p b r (a2 s)", a2=4)
        for bi, dst in ((0, hc_s), (1, hd_s)):
            hr = en.sync.dma_start(
                dst[:].rearrange("p (r a2) s -> p r (a2 s)", a2=4),
                h_src[:, bi])
            tile.add_dep_helper(hr.ins, h_cc.ins, reason="h a2a done")
        # gp := hc + hd; gc := gelu(hc) -> hd_s; gelu(gp) -> hc_s;
        # gd := gelu(hp) - gc -> gp_s.  (runs during the y a2a wait)
        en.vector.tensor_tensor(gp_s[:], hc_s[:], hd_s[:], op=OP.add)
        en.scalar.activation(hd_s[:], hc_s[:], AF.Gelu)
        en.scalar.activation(hc_s[:], gp_s[:], AF.Gelu)
        en.vector.tensor_sub(gp_s[:], hc_s[:], hd_s[:])
        a_s = hc_s        # gelu(hp) content is dead once gd is computed

        # PE warm-keeper: a short matmul burst after gate2 drains plus a
        # couple per search round keeps the HAM clock gate open so the
        # down projection runs at full rate (outputs unused)
        pwm = ps.tile([128, 512], f32, tag="ps", name="pwm")

        def warm_mms(n):
            for _ in range(n):
                en.tensor.matmul(pwm[:], up_s["c"][:, 0, 0:128],
                                 up_s["c"][:, 0, :], start=True, stop=True)

        warm_mms(8)

        # ---------- y16 fill (two parallel DMA queues) ----------
        y16 = big.tile([128, KT, S], fp16, tag="y16", name="y16")
        yfills = []
        for hh, eng in ((0, en.sync), (1, en.scalar)):
            yf = eng.dma_start(
                y16[:, hh * 16:(hh + 1) * 16, :]
                .rearrange("p (r a2) s -> p r (a2 s)", a2=4),
                y_a2a_out[hh * 4:(hh + 1) * 4]
                .rearrange("r (p a2) s -> p r (a2 s)", a2=4))
            tile.add_dep_helper(yf.ins, y_cc.ins, reason="y a2a done")
            yfills.append(yf)

        # ---------- Newton threshold search on y16 ----------
        y_flat = y16.rearrange("p a b -> p (a b)")
        half = (KT * S) // 2
        # scratch targets for the count passes: carve them out of the
        # long-dead xh tile (count writes are garbage, only accum matters)
        cmpb = xh_s[:, 0:1, :].bitcast(fp8e4).rearrange("p a t -> p (a t)")
        sgnb = xh_s[:, 2:3, :].bitcast(fp8e4).rearrange("p a t -> p (a t)")
        ones32 = const.tile([128, 128], f32, tag="ones32", name="ones32")
        en.vector.memset(ones32[:], 1.0)
        yt = const.tile([128, 1], f32, tag="yt", name="yt")
        nyt = const.tile([128, 1], f32, tag="nyt", name="nyt")
        uu = const.tile([128, 1], f32, tag="uu", name="uu")
        en.vector.memset(yt[:], 0.0)
        en.vector.memset(nyt[:], 0.0)
        accs = const.tile([128, 2], f32, tag="accs", name="accs")
        rmax = const.tile([128, KT], f32, tag="rmax", name="rmax")

        for r in range(R_ITER):
            hv = en.vector.tensor_scalar(cmpb, y_flat[:, 0:half],
                                         yt[:], 0.0, op0=OP.is_gt,
                                         op1=OP.add, accum_out=accs[:, 0:1])
            hs = en.scalar.activation(sgnb, y_flat[:, half:],
                                      AF.Sign, bias=nyt[:],
                                      accum_out=accs[:, 1:2])
            if r == 0:
                for dep in yfills:
                    tile.add_dep_helper(hv.ins, dep.ins, reason="y16 ready")
                    tile.add_dep_helper(hs.ins, dep.ins, reason="y16 ready")
            en.vector.scalar_tensor_tensor(uu[:], accs[:, 1:2], 0.5,
                                           accs[:, 0:1],
                                           op0=OP.mult, op1=OP.add)
            en.vector.tensor_scalar(uu[:], uu[:], 768.0, None, op0=OP.add)
            pred = ps.tile([128, 1], f32, tag="ps", name=f"pred{r}")
            en.tensor.matmul(pred[:], ones32[:], uu[:],
                             start=True, stop=True)
            yt_h = en.vector.scalar_tensor_tensor(yt[:], pred[:], C_NEWTON,
                                                  yt[:],
                                                  op0=OP.mult, op1=OP.add)
            if r == 0:
                round0_yt = yt_h
            en.vector.tensor_scalar(nyt[:], yt[:], -1.0, None, op0=OP.mult)
            warm_mms(3)
            if r == 1:
                # rowmax (needed only for the final select) slots into
                # vector gaps between rounds; pin it after round 0 so the
                # scheduler cannot hoist it ahead of the first count
                for q in range(4):
                    hq_ = en.vector.reduce_max(rmax[:, q * 8:(q + 1) * 8],
                                               y16[:, q * 8:(q + 1) * 8, :],
                                               axis=mybir.AxisListType.X)
                    tile.add_dep_helper(hq_.ins, round0_yt.ins,
                                        reason="defer rowmax past round 0")

        sel = const.tile([128, KT], f32, tag="sel", name="sel")
        en.vector.tensor_scalar(sel[:], rmax[:], yt[:], None, op0=OP.is_gt)

        # ---------- select + down, pipelined per 8-ktile chunk ----------
        # a := gd*sel + gc  (gd in gp_s, gc in hd_s) -> a_s in hc_s
        pd0 = ps.tile([128, 512], f32, tag="ps", name="dn0")
        pd1 = ps.tile([128, 512], f32, tag="ps", name="dn1")
        for q in range(4):
            issue_dw(8)
            for kt in range(q * 8, (q + 1) * 8):
                en.vector.scalar_tensor_tensor(
                    a_s[:, kt, :], gp_s[:, kt, :], sel[:, kt:kt + 1],
                    hd_s[:, kt, :], op0=OP.mult, op1=OP.add)
            for kt in range(q * 8, (q + 1) * 8):
                dwk = dw_tiles[kt]
                en.tensor.matmul(pd0[:], a_s[:, kt, :], dwk[:, N0],
                                 start=(kt == 0), stop=(kt == KT - 1))
                en.tensor.matmul(pd1[:], a_s[:, kt, :], dwk[:, N1],
                                 start=(kt == 0), stop=(kt == KT - 1))
        osb = drain.tile([128, D], f32, tag="gf", name="osb")
        en.vector.tensor_tensor(osb[:, N0], pd0[:], dbias_s[:, N0], op=OP.add)
        en.sync.dma_start(out_d[:, N0], osb[:, N0])
        en.vector.tensor_tensor(osb[:, N1], pd1[:], dbias_s[:, N1], op=OP.add)
        en.sync.dma_start(out_d[:, N1], osb[:, N1])

    nc.compile()
    return nc


_NC_CACHE = None


def _f16hi(a, scale):
    """fp16(a*scale) and the fp32 residual a - fp16(a*scale)/scale."""
    hi = (a.astype(np.float64) * scale).astype(F16)
    res = (a.astype(np.float64) - hi.astype(np.float64) / scale).astype(F32)
    return hi, res


def _pair_k(a):
    """[n*256, X] -> [n, 128, 2, X] : (2t, 2t+1) k-tile pairs."""
    n = a.shape[0] // 256
    return np.ascontiguousarray(a.reshape(n, 2, 128, -1).transpose(0, 2, 1, 3))


def _prep_in_maps(x, w1, b1, w2, b2, upw, upb, ucw, ucb, dw, db):
    xt = np.ascontiguousarray(x.reshape(T, D).T).astype(F32)   # [D, T]
    xh16 = xt.astype(F16)
    x_lo = xt - xh16.astype(F32)
    x8a = _pair_k((xt * 0.5).astype(FP8))
    x8b = _pair_k((x_lo * (2.0 ** 9)).astype(FP8))
    udw = upw - ucw
    udb = upb - ucb
    dwT16 = np.ascontiguousarray(dw.T.astype(F16))             # [H, D]
    dbias = np.ascontiguousarray(np.tile(db[None, :], (128, 1)).astype(F32))

    perm = (np.asarray(KT_ORDER)[:, None] * 128
            + np.arange(128)[None, :]).reshape(-1)

    in_maps = []
    for c in range(NCORES):
        sh = slice(c * HS, (c + 1) * HS)
        w1t = np.ascontiguousarray(w1[sh].T).astype(F32)       # [D, HS]
        w1h, w1res = _f16hi(w1t, 2.0 ** 15)
        w2t = np.ascontiguousarray(w2[sh].T).astype(F32)       # [H, HS]
        w2t = np.ascontiguousarray(w2t[perm])
        w2h, w2res = _f16hi(w2t, 2.0 ** 15)
        b2y = (b2[sh].astype(F32) - Y_OFF) * Y_SCL
        in_maps.append({
            "xh16": xh16,
            "x8a": x8a,
            "x8b": x8b,
            "w1h16": w1h,
            "w1a8": _pair_k((w1res * (2.0 ** 16)).astype(FP8)),
            "w1b8": _pair_k((w1t * (2.0 ** 6)).astype(FP8)),
            "w2h16": np.ascontiguousarray(w2h.reshape(KT, 128, HS)),
            "w2a8": _pair_k((w2res * (2.0 ** 16)).astype(FP8)),
            "w2b8": _pair_k((w2t * (2.0 ** 3)).astype(FP8)),
            "upc16": np.ascontiguousarray(ucw[sh].T.astype(F16)),
            "upd16": np.ascontiguousarray(udw[sh].T.astype(F16)),
            "dwT16": dwT16,
            "b1s": np.ascontiguousarray(b1[sh].reshape(4, 128)).astype(F32),
            "b2ys": np.ascontiguousarray(b2y.reshape(4, 128)).astype(F32),
            "bcs": np.ascontiguousarray(ucb[sh].reshape(4, 128)).astype(F32),
            "bds": np.ascontiguousarray(udb[sh].reshape(4, 128)).astype(F32),
            "dbias": dbias,
        })
    return in_maps


def kernel_in_maps(**inputs):
    names = ["inputs", "gate_w1", "gate_b1", "gate_w2", "gate_b2",
             "up_prev_w", "up_prev_b", "up_curr_w", "up_curr_b",
             "down_w", "down_b"]
    vals = [np.asarray(inputs[n], F32) for n in names]
    return _prep_in_maps(*vals)


def kernel(**inputs):
    global _NC_CACHE
    if _NC_CACHE is None:
        _NC_CACHE = _build()
    nc = _NC_CACHE
    in_maps = kernel_in_maps(**inputs)
    res = run_bass_kernel_spmd(nc, in_maps, core_ids=list(range(NCORES)))
    out = np.stack([res.results[c]["out"] for c in range(NCORES)], axis=0)
    return np.ascontiguousarray(out.astype(F32))
